# revision 14
# baseline (speedup 1.0000x reference)
"""Trainium2 Bass kernel for cumulative-state (linear) attention over M modalities.

Math (reference): out[i, e] = sum_m sum_{j : t2_m[j] <= t1[i]} (Q[i] . K_m[j]) * X_m[j, e],
for e in {0, 1}, where Q = mlp_q(X[0]), K_m = mlp_km(X[m]), t1 = X[0,:,-1], t2_m = X[m,:,-1].

Sharding: 8 cores = (m, h): modality m in 0..3, key-half h in 0..1. Each core owns
keys j in [h*4096, (h+1)*4096) of modality m and computes partial contributions for
ALL queries; the host scatter-sums the 8 partial outputs (the "all-reduce").

Per core the key range is split into 32 chunks of 128. Query i with
idx[i] = searchsorted(t2_m, t1[i], right) - 1 receives this core's contribution as
  Q[i] @ S_run(k)  +  sum_{j in chunk k, t2[j] <= t1[i]} (Q[i].K[j]) V[j,:2]
with k the local chunk containing idx[i], S_run(k) = sum over local chunks < k of
K^T V2; queries past the range get the tail term Q[i] @ S_run(32). Band boundaries
and the (exact, integer) causal mask come from host searchsorted and are baked into
the single SPMD-shared static graph; per-core variability lives entirely in data.

All 64-contract work is ROW-PACKED: activations live as (128, n/2) tiles whose
partition halves hold two independent column blocks, processed by concurrent
tile_position=(0,0)/(64,64) matmul pairs and full-width DVE/ACT epilogues. The 32
key chunks are split into two 16-chunk sides so each band chunk's K-half matches
its query-band half. Matmul operands are bf16 (PSUM accumulation stays f32).
"""

import os
from contextlib import ExitStack

import ml_dtypes
import numpy as np

BF16 = ml_dtypes.bfloat16

M, T, D = 4, 8192, 64
NLIN = 3
C = 128          # key chunk size
NK = T // 2      # keys per core (4096)
NKC = NK // C    # local key chunks per core (32)
NSIDE = NKC // 2
NCORES = 8
FMAX = 512       # max matmul free dim / PSUM bank cols (f32)


def _round_up(x, k):
    return ((x + k - 1) // k) * k


def make_plan(X):
    """Host-side: band structure + packed column layout, shared across cores."""
    X = np.asarray(X, np.float32)
    t1 = X[0, :, -1]
    los, his, tbs, idxs = [], [], [], []
    for c in range(NCORES):
        m, h = c // 2, c % 2
        t2 = X[m, :, -1]
        idx = np.searchsorted(t2, t1, side="right") - 1
        idxs.append(idx)
        hs = h * NK
        lo = np.searchsorted(idx, hs + np.arange(NKC) * C, side="left")
        hi = np.searchsorted(idx, hs + (np.arange(NKC) + 1) * C, side="left")
        los.append(lo)
        his.append(hi)
        tbs.append(int(np.searchsorted(idx, hs + NK, side="left")))

    NB = [0] * NKC
    for k in range(NKC):
        w = max(his[c][k] - los[c][k] for c in range(NCORES))
        NB[k] = _round_up(int(w), 8)
    NBAND = int(sum(NB))
    NT = _round_up(max(T - tb for tb in tbs), 8)

    # split 32 chunks into two 16-chunk sides with balanced band totals
    order = sorted(range(NKC), key=lambda k: -NB[k])
    sideof = [0] * NKC
    tot = [0, 0]
    cnt = [0, 0]
    for k in order:
        s = 0 if (tot[0] <= tot[1] and cnt[0] < NSIDE) or cnt[1] >= NSIDE else 1
        sideof[k] = s
        tot[s] += NB[k]
        cnt[s] += 1
    lb, rb = tot[0], tot[1]
    # tail split across sides to balance
    tL = int(np.clip(_round_up((NBAND + NT) // 2 - lb, 8), 0, NT))
    tR = NT - tL
    NW2 = max(lb + tL, rb + tR)

    kpos = [0] * NKC
    qoff = [0] * NKC
    acc = [0, 0]
    pos = [0, 0]
    for k in range(NKC):  # global ascending within each side
        s = sideof[k]
        kpos[k] = pos[s]
        qoff[k] = acc[s]
        pos[s] += 1
        acc[s] += NB[k]
    toff = [lb, rb]  # tail start col within each side
    tlen = [tL, tR]

    return dict(NB=NB, NBAND=NBAND, NT=NT, NW2=NW2, sideof=sideof, kpos=kpos,
                qoff=qoff, toff=toff, tlen=tlen, los=los, his=his, tbs=tbs,
                idxs=idxs)


def make_inputs(X, wq_w, wq_b, wk_w, wk_b, plan):
    X = np.asarray(X, np.float32)
    wq_w = np.asarray(wq_w, np.float32)
    wq_b = np.asarray(wq_b, np.float32)
    wk_w = np.asarray(wk_w, np.float32)
    wk_b = np.asarray(wk_b, np.float32)
    NB, NW2 = plan["NB"], plan["NW2"]
    sideof, kpos, qoff = plan["sideof"], plan["kpos"], plan["qoff"]
    toff, tlen = plan["toff"], plan["tlen"]

    # weights stacked into both partition halves
    wq1 = np.concatenate([wq_w[l] for l in range(NLIN)], axis=1)
    wq = np.concatenate([wq1, wq1], axis=0).astype(BF16)              # (128, 192)
    bq1 = np.stack([wq_b[l] for l in range(NLIN)], axis=1)
    bq = np.concatenate([bq1, bq1], axis=0).astype(np.float32)        # (128, 3)

    in_maps = []
    for c in range(NCORES):
        m, h = c // 2, c % 2
        hs = h * NK
        lo, hi, tb = plan["los"][c], plan["his"][c], plan["tbs"][c]
        idx = plan["idxs"][c]

        qb = np.zeros((2 * D, NW2), BF16)
        msk = np.zeros((C, 2 * NW2), BF16)
        for k in range(NKC):
            n = hi[k] - lo[k]
            s, o = sideof[k], qoff[k]
            if n > 0:
                qb[64 * s:64 * s + 64, o:o + n] = X[0, lo[k]:hi[k], :].T.astype(BF16)
                jg = hs + k * C + np.arange(C)[:, None]
                msk[:, s * NW2 + o:s * NW2 + o + n] = \
                    (jg <= idx[None, lo[k]:hi[k]]).astype(BF16)
        # tail: first tlen[0] tail queries on side 0, rest on side 1
        ntail = T - tb
        n0 = min(ntail, tlen[0])
        if n0 > 0:
            qb[0:64, toff[0]:toff[0] + n0] = X[0, tb:tb + n0, :].T.astype(BF16)
        n1 = ntail - n0
        if n1 > 0:
            qb[64:128, toff[1]:toff[1] + n1] = X[0, tb + n0:, :].T.astype(BF16)

        xk = X[m, hs:hs + NK, :]
        xkt = np.zeros((2 * D, NSIDE * C), BF16)
        v2 = np.zeros((C, 2 * NKC), BF16)
        for k in range(NKC):
            s, p = sideof[k], kpos[k]
            xkt[64 * s:64 * s + 64, p * C:(p + 1) * C] = \
                xk[k * C:(k + 1) * C, :].T.astype(BF16)
            v2[:, 2 * k:2 * k + 2] = xk[k * C:(k + 1) * C, 0:2].astype(BF16)

        wk1 = np.concatenate([wk_w[m, l] for l in range(NLIN)], axis=1)
        wk = np.concatenate([wk1, wk1], axis=0).astype(BF16)          # (128, 192)
        bk1 = np.stack([wk_b[m, l] for l in range(NLIN)], axis=1)
        bk = np.concatenate([bk1, bk1], axis=0).astype(np.float32)    # (128, 3)

        # host-computed S correction: the last K-linear's bias contributes
        # b3 (x) sum_j v2[j,:] per chunk — fold it into the sc prefix inputs
        b3 = wk_b[m, NLIN - 1]                                        # (64,)
        scorr1 = np.zeros((D, 2 * NKC), np.float32)
        for k in range(NKC):
            vs = np.asarray(v2[:, 2 * k:2 * k + 2], np.float32).sum(axis=0)  # (2,)
            scorr1[:, 2 * k:2 * k + 2] = b3[:, None] * vs[None, :]
        scorr = np.concatenate([scorr1, scorr1], axis=0)              # (128, 64)

        in_maps.append(dict(qb=qb, msk=msk, xkt=xkt, v2=v2,
                            wq=wq, bq=bq, wk=wk, bk=bk, scorr=scorr))
    return in_maps


def scatter_outputs(plan, outs):
    """Host-side 'all-reduce': scatter per-core (2, 2*NW2) partials to (T, 2)."""
    NB, NW2 = plan["NB"], plan["NW2"]
    sideof, qoff, toff, tlen = plan["sideof"], plan["qoff"], plan["toff"], plan["tlen"]
    y = np.zeros((T, 2), np.float32)
    for c in range(NCORES):
        o = np.asarray(outs[c], np.float32)
        lo, hi, tb = plan["los"][c], plan["his"][c], plan["tbs"][c]
        for k in range(NKC):
            n = hi[k] - lo[k]
            if n > 0:
                base = sideof[k] * NW2 + qoff[k]
                y[lo[k]:hi[k], :] += o[:, base:base + n].T
        ntail = T - tb
        n0 = min(ntail, tlen[0])
        if n0 > 0:
            y[tb:tb + n0, :] += o[:, toff[0]:toff[0] + n0].T
        n1 = ntail - n0
        if n1 > 0:
            y[tb + n0:, :] += o[:, NW2 + toff[1]:NW2 + toff[1] + n1].T
    return y


# ---------------------------------------------------------------- numpy emulation
def emulate_core(im, plan):
    """Numpy mirror of the device graph for one core (f32 math, for validation)."""
    NB, NW2 = plan["NB"], plan["NW2"]
    sideof, kpos, qoff = plan["sideof"], plan["kpos"], plan["qoff"]
    toff, tlen = plan["toff"], plan["tlen"]

    def f(x):
        return np.asarray(x, np.float32)

    wk, bk, wq, bq = f(im["wk"]), f(im["bk"]), f(im["wq"]), f(im["bq"])
    qb, xkt, v2, msk = f(im["qb"]), f(im["xkt"]), f(im["v2"]), f(im["msk"])

    def mlp_packed(xp, w, b):
        """Row-packed MLP: both halves with their own (identical) weights."""
        a = xp
        outs = []
        for l in range(NLIN):
            z = np.concatenate([
                w[0:64, 64 * l:64 * (l + 1)].T @ a[0:64] + b[0:64, l][:, None],
                w[64:128, 64 * l:64 * (l + 1)].T @ a[64:128] + b[64:128, l][:, None],
            ], axis=0)
            a = np.maximum(z, 0.0) if l < NLIN - 1 else z
            outs.append(a)
        return outs[-1], outs[-2]

    ktp, a2p = mlp_packed(xkt, wk, bk)       # (128, NSIDE*C)
    qtp, _ = mlp_packed(qb, wq, bq)          # (128, NW2)

    # K natural (bias-free) + S chunks (global order) + host bias correction
    sc = np.zeros((64, 2 * NKC), np.float32)
    for k in range(NKC):
        s, p = sideof[k], kpos[k]
        a2 = a2p[64 * s:64 * s + 64, p * C:(p + 1) * C]
        kn = a2.T @ wk[64 * s:64 * s + 64, 128:192]
        sc[:, 2 * k:2 * k + 2] = kn.T @ v2[:, 2 * k:2 * k + 2]
    sc = sc + f(im["scorr"])[0:64]
    srun = np.zeros((64, 2 * (NKC + 1)), np.float32)
    for k in range(NKC):
        srun[:, 2 * k + 2:2 * k + 4] = srun[:, 2 * k:2 * k + 2] + sc[:, 2 * k:2 * k + 2]

    out = np.zeros((2, 2 * NW2), np.float32)
    for k in range(NKC):
        nq = NB[k]
        s, p, o = sideof[k], kpos[k], qoff[k]
        qblk = qtp[64 * s:64 * s + 64, o:o + nq]
        mask = msk[:, s * NW2 + o:s * NW2 + o + nq]
        B = ktp[64 * s:64 * s + 64, p * C:(p + 1) * C].T @ qblk
        out[:, s * NW2 + o:s * NW2 + o + nq] = (
            srun[:, 2 * k:2 * k + 2].T @ qblk + v2[:, 2 * k:2 * k + 2].T @ (B * mask))
    for s in range(2):
        if plan["tlen"][s] > 0:
            o = toff[s]
            out[:, s * NW2 + o:s * NW2 + o + tlen[s]] = \
                srun[:, 2 * NKC:2 * NKC + 2].T @ qtp[64 * s:64 * s + 64, o:o + tlen[s]]
    return out


# ---------------------------------------------------------------- device graph
def build_graph(plan):
    import concourse.bacc as bacc
    import concourse.tile as tile
    from concourse import mybir

    NB, NW2, NT = plan["NB"], plan["NW2"], plan["NT"]
    sideof, kpos, qoff = plan["sideof"], plan["kpos"], plan["qoff"]
    toff, tlen = plan["toff"], plan["tlen"]
    f32 = mybir.dt.float32
    bf16 = mybir.dt.bfloat16
    AF = mybir.ActivationFunctionType
    OP = mybir.AluOpType
    KW = NSIDE * C  # 2048 key cols per side

    nc = bacc.Bacc("TRN2")
    d_qb = nc.dram_tensor("qb", [2 * D, NW2], bf16, kind="ExternalInput")
    d_msk = nc.dram_tensor("msk", [C, 2 * NW2], bf16, kind="ExternalInput")
    d_xkt = nc.dram_tensor("xkt", [2 * D, KW], bf16, kind="ExternalInput")
    d_v2 = nc.dram_tensor("v2", [C, 2 * NKC], bf16, kind="ExternalInput")
    d_wq = nc.dram_tensor("wq", [2 * D, D * NLIN], bf16, kind="ExternalInput")
    d_bq = nc.dram_tensor("bq", [2 * D, NLIN], f32, kind="ExternalInput")
    d_wk = nc.dram_tensor("wk", [2 * D, D * NLIN], bf16, kind="ExternalInput")
    d_bk = nc.dram_tensor("bk", [2 * D, NLIN], f32, kind="ExternalInput")
    d_scorr = nc.dram_tensor("scorr", [C, 2 * NKC], f32, kind="ExternalInput")
    d_out = nc.dram_tensor("out", [2, 2 * NW2], f32, kind="ExternalOutput")

    with ExitStack() as ctx:
        tc = ctx.enter_context(tile.TileContext(nc))
        const = ctx.enter_context(tc.tile_pool(name="const", bufs=1))
        big = ctx.enter_context(tc.tile_pool(name="big", bufs=1))
        work = ctx.enter_context(tc.tile_pool(name="work", bufs=3))
        pmlp = ctx.enter_context(tc.tile_pool(name="pmlp", bufs=3, space="PSUM"))
        ps = ctx.enter_context(tc.tile_pool(name="ps", bufs=1, space="PSUM"))
        pb = ctx.enter_context(tc.tile_pool(name="pb", bufs=2, space="PSUM"))
        pout = ctx.enter_context(tc.tile_pool(name="pout", bufs=2, space="PSUM"))

        wq_t = const.tile([2 * D, D * NLIN], bf16, tag="wq")
        bq_t = const.tile([2 * D, NLIN], f32, tag="bq")
        wk_t = const.tile([2 * D, D * NLIN], bf16, tag="wk")
        bk_t = const.tile([2 * D, NLIN], f32, tag="bk")
        scorr_t = const.tile([C, 2 * NKC], f32, tag="scorr")
        v2_t = const.tile([C, 2 * NKC], bf16, tag="v2")

        # compute-critical loads first, spread over the two HWDGE engines
        nc.sync.dma_start(wk_t[:], d_wk[:])
        nc.scalar.dma_start(bk_t[:], d_bk[:])
        xkt_t = big.tile([2 * D, KW], bf16, tag="xkt")
        nc.sync.dma_start(xkt_t[:], d_xkt[:])
        nc.scalar.dma_start(wq_t[:], d_wq[:])
        nc.scalar.dma_start(bq_t[:], d_bq[:])
        qb_t = big.tile([2 * D, NW2], bf16, tag="qb")
        NQB = 4
        qsp = _round_up((NW2 + NQB - 1) // NQB, 8)
        for i in range(NQB):
            a, b = i * qsp, min((i + 1) * qsp, NW2)
            if a < b:
                nc.sync.dma_start(qb_t[:, a:b], d_qb[:, a:b])
        nc.scalar.dma_start(v2_t[:], d_v2[:])
        nc.scalar.dma_start(scorr_t[:], d_scorr[:])
        msk_t = big.tile([C, 2 * NW2], bf16, tag="msk")
        NMQ = 4
        msp = _round_up((2 * NW2 + NMQ - 1) // NMQ, 8)
        for i in range(NMQ):
            a, b = i * msp, min((i + 1) * msp, 2 * NW2)
            if a < b:
                nc.scalar.dma_start(msk_t[:, a:b], d_msk[:, a:b])

        kt_t = big.tile([2 * D, KW], bf16, tag="kt")
        a2k_t = big.tile([2 * D, KW], bf16, tag="a2k")
        qt_t = big.tile([2 * D, NW2], bf16, tag="qt")
        kn_t = big.tile([C, D * NKC], bf16, tag="kn")
        sc_t = big.tile([C, 2 * NKC], f32, tag="sc")
        srun_t = big.tile([C, 2 * (NKC + 1)], f32, tag="srun")
        srunb_t = big.tile([C, 2 * (NKC + 1)], bf16, tag="srunb")
        outs_t = big.tile([2, 2 * NW2], f32, tag="outs")

        # PE warm-up burst: ~4us of dense dummy matmuls during the initial DMA
        # window flips the HAM clock gate to 8/8 before real work arrives
        wup_t = work.tile([C, FMAX], bf16, tag="wup", name="wup")
        nc.vector.memset(wup_t[:], 0.0)
        for _ in range(12):
            pwu = pb.tile([C, FMAX], f32, tag="pb", name="pwu")
            nc.tensor.matmul(pwu[:], wup_t[:, 0:C], wup_t[:], start=True, stop=True)

        def mlp3(src_t, w_t, b_t, n_cols, out_t, mid_t, eng):
            """Row-packed 3-layer MLP: concurrent (0,0)/(64,64) matmul pairs."""
            for a in range(0, n_cols, FMAX):
                b = min(a + FMAX, n_cols)
                n = b - a
                cur = src_t[:, a:b]
                for l in range(NLIN):
                    pz = pmlp.tile([C, FMAX], f32, tag="pmlp", name="pz")
                    nc.tensor.matmul(pz[0:64, :n], w_t[0:64, D * l:D * (l + 1)],
                                     cur[0:64, :], start=True, stop=True,
                                     tile_position=(0, 0))
                    nc.tensor.matmul(pz[64:128, :n], w_t[64:128, D * l:D * (l + 1)],
                                     cur[64:128, :], start=True, stop=True,
                                     tile_position=(64, 64))
                    if l < NLIN - 1:
                        dst = (mid_t[:, a:b] if l == NLIN - 2
                               else work.tile([C, FMAX], bf16, tag="mlpa",
                                              name="mlpa")[:, :n])
                    else:
                        dst = out_t[:, a:b]
                    if eng == "act":
                        nc.scalar.activation(dst, pz[:, :n],
                                             AF.Relu if l < NLIN - 1 else AF.Identity,
                                             bias=b_t[:, l:l + 1])
                    else:
                        if l < NLIN - 1:
                            nc.vector.tensor_scalar(dst, pz[:, :n],
                                                    b_t[:, l:l + 1], 0.0,
                                                    OP.add, OP.max)
                        else:
                            nc.vector.tensor_scalar_add(dst, pz[:, :n],
                                                        b_t[:, l:l + 1])
                    cur = dst

        # K mlp (ACT epilogues)
        mlp3(xkt_t, wk_t, bk_t, KW, kt_t, a2k_t, "act")

        # K natural per chunk (bias-free; bias folded into scorr): full 128-part out
        for k in range(NKC):
            s, p = sideof[k], kpos[k]
            pkn = pb.tile([C, FMAX], f32, tag="pb", name="pkn")
            nc.tensor.matmul(pkn[:, :D], a2k_t[64 * s:64 * s + 64, C * p:C * (p + 1)],
                             wk_t[64 * s:64 * s + 64, 2 * D:3 * D],
                             start=True, stop=True, tile_position=(64 * s, 0))
            nc.scalar.copy(kn_t[:, D * k:D * (k + 1)], pkn[:, :D])

        # S chunks, duplicated into both partition halves of one PSUM tile
        psc = ps.tile([C, 2 * NKC], f32, tag="ps", name="psc")
        for k in range(NKC):
            nc.tensor.matmul(psc[0:64, 2 * k:2 * k + 2], kn_t[:, D * k:D * (k + 1)],
                             v2_t[:, 2 * k:2 * k + 2], start=True, stop=True,
                             tile_position=(0, 0))
            nc.tensor.matmul(psc[64:128, 2 * k:2 * k + 2], kn_t[:, D * k:D * (k + 1)],
                             v2_t[:, 2 * k:2 * k + 2], start=True, stop=True,
                             tile_position=(0, 64))
        nc.vector.tensor_add(sc_t[:], psc[:], scorr_t[:])

        # prefix sums (f32, both halves at once) + bf16 copy for lhsT use
        nc.vector.memset(srun_t[:, 0:2], 0.0)
        for k in range(NKC):
            nc.vector.tensor_add(srun_t[:, 2 * k + 2:2 * k + 4],
                                 srun_t[:, 2 * k:2 * k + 2], sc_t[:, 2 * k:2 * k + 2])
        nc.scalar.copy(srunb_t[:], srun_t[:])

        # Q mlp (DVE epilogues; mid shares qt_t — Tile serializes the WAR)
        mlp3(qb_t, wq_t, bq_t, NW2, qt_t, qt_t, "dve")

        # band chunks, ordered by (side, column offset) for locality
        korder = sorted(range(NKC), key=lambda k: (sideof[k], qoff[k]))
        for k in korder:
            nq = NB[k]
            s, p, o0 = sideof[k], kpos[k], int(qoff[k])
            for a in range(0, nq, FMAX):
                b = min(a + FMAX, nq)
                n = b - a
                qs = qt_t[64 * s:64 * s + 64, o0 + a:o0 + b]
                pB = pb.tile([C, FMAX], f32, tag="pb", name="pB")
                nc.tensor.matmul(pB[:, :n], kt_t[64 * s:64 * s + 64, C * p:C * (p + 1)],
                                 qs, start=True, stop=True, tile_position=(64 * s, 0))
                bm = work.tile([C, FMAX], bf16, tag="bm", name="bm")
                nc.vector.tensor_mul(bm[:, :n], pB[:, :n],
                                     msk_t[:, s * NW2 + o0 + a:s * NW2 + o0 + b])
                po = pout.tile([2, FMAX], f32, tag="pout", name="po")
                nc.tensor.matmul(po[:, :n], srunb_t[64 * s:64 * s + 64, 2 * k:2 * k + 2],
                                 qs, start=True, stop=False, tile_position=(64 * s, 0))
                nc.tensor.matmul(po[:, :n], v2_t[:, 2 * k:2 * k + 2], bm[:, :n],
                                 start=False, stop=True, tile_position=(0, 0))
                nc.scalar.copy(outs_t[:, s * NW2 + o0 + a:s * NW2 + o0 + b], po[:, :n])

        # tails (state-only), per side
        for s in range(2):
            for a in range(0, tlen[s], FMAX):
                b = min(a + FMAX, tlen[s])
                n = b - a
                po = pout.tile([2, FMAX], f32, tag="pout", name="po_t")
                nc.tensor.matmul(po[:, :n],
                                 srunb_t[64 * s:64 * s + 64, 2 * NKC:2 * NKC + 2],
                                 qt_t[64 * s:64 * s + 64, toff[s] + a:toff[s] + b],
                                 start=True, stop=True, tile_position=(64 * s, 0))
                nc.scalar.copy(outs_t[:, s * NW2 + toff[s] + a:s * NW2 + toff[s] + b],
                               po[:, :n])

        nc.sync.dma_start(d_out[:], outs_t[:])

    nc.finalize()
    return nc


_CACHE = {}


def kernel(X, wq_w, wq_b, wk_w, wk_b):
    from concourse.bass_utils import run_bass_kernel_spmd

    plan = make_plan(X)
    in_maps = make_inputs(X, wq_w, wq_b, wk_w, wk_b, plan)
    key = (tuple(plan["NB"]), plan["NT"], tuple(plan["sideof"]))
    if key not in _CACHE:
        _CACHE[key] = build_graph(plan)
    nc = _CACHE[key]
    res = run_bass_kernel_spmd(nc, in_maps, core_ids=list(range(NCORES)),
                               trace=bool(int(os.environ.get("KTRACE", "0"))))
    outs = [res.results[c]["out"] for c in range(NCORES)]
    y = scatter_outputs(plan, outs)
    if os.environ.get("KTRACE", "0") != "0":
        kernel.last_result = res
    return y[None]  # (1, T, 2)


# revision 16
# speedup vs baseline: 1.1162x; 1.1162x over previous
"""Trainium2 Bass kernel for cumulative-state (linear) attention over M modalities.

Math (reference): out[i, e] = sum_m sum_{j : t2_m[j] <= t1[i]} (Q[i] . K_m[j]) * X_m[j, e],
for e in {0, 1}, where Q = mlp_q(X[0]), K_m = mlp_km(X[m]), t1 = X[0,:,-1], t2_m = X[m,:,-1].

Sharding: 8 cores = (m, h): modality m in 0..3, key-half h in 0..1. Each core owns
keys j in [h*4096, (h+1)*4096) of modality m and computes partial contributions for
ALL queries; the host scatter-sums the 8 partial outputs (the "all-reduce").

Per core the key range is split into 32 chunks of 128. Query i with
idx[i] = searchsorted(t2_m, t1[i], right) - 1 receives this core's contribution as
  Q[i] @ S_run(k)  +  sum_{j in chunk k, t2[j] <= t1[i]} (Q[i].K[j]) V[j,:2]
with k the local chunk containing idx[i], S_run(k) = sum over local chunks < k of
K^T V2; queries past the range get the tail term Q[i] @ S_run(32). Band boundaries
and the (exact, integer) causal mask come from host searchsorted and are baked into
the single SPMD-shared static graph; per-core variability lives entirely in data.

All 64-contract work is ROW-PACKED: activations live as (128, n/2) tiles whose
partition halves hold two independent column blocks, processed by concurrent
tile_position=(0,0)/(64,64) matmul pairs and full-width DVE/ACT epilogues. The 32
key chunks are split into two 16-chunk sides so each band chunk's K-half matches
its query-band half. Matmul operands are bf16 (PSUM accumulation stays f32).
"""

import os
from contextlib import ExitStack

import ml_dtypes
import numpy as np

BF16 = ml_dtypes.bfloat16

M, T, D = 4, 8192, 64
NLIN = 3
C = 128          # key chunk size
NK = T // 2      # keys per core (4096)
NKC = NK // C    # local key chunks per core (32)
NSIDE = NKC // 2
NCORES = 8
FMAX = 512       # max matmul free dim / PSUM bank cols (f32)


def _round_up(x, k):
    return ((x + k - 1) // k) * k


def make_plan(X):
    """Host-side: band structure + packed column layout, shared across cores."""
    X = np.asarray(X, np.float32)
    t1 = X[0, :, -1]
    los, his, tbs, idxs = [], [], [], []
    for c in range(NCORES):
        m, h = c // 2, c % 2
        t2 = X[m, :, -1]
        idx = np.searchsorted(t2, t1, side="right") - 1
        idxs.append(idx)
        hs = h * NK
        lo = np.searchsorted(idx, hs + np.arange(NKC) * C, side="left")
        hi = np.searchsorted(idx, hs + (np.arange(NKC) + 1) * C, side="left")
        los.append(lo)
        his.append(hi)
        tbs.append(int(np.searchsorted(idx, hs + NK, side="left")))

    NB = [0] * NKC
    for k in range(NKC):
        w = max(his[c][k] - los[c][k] for c in range(NCORES))
        NB[k] = _round_up(int(w), 8)
    NBAND = int(sum(NB))
    NT = _round_up(max(T - tb for tb in tbs), 8)

    # split 32 chunks into two 16-chunk sides with balanced band totals
    order = sorted(range(NKC), key=lambda k: -NB[k])
    sideof = [0] * NKC
    tot = [0, 0]
    cnt = [0, 0]
    for k in order:
        s = 0 if (tot[0] <= tot[1] and cnt[0] < NSIDE) or cnt[1] >= NSIDE else 1
        sideof[k] = s
        tot[s] += NB[k]
        cnt[s] += 1
    lb, rb = tot[0], tot[1]
    # tail split across sides to balance
    tL = int(np.clip(_round_up((NBAND + NT) // 2 - lb, 8), 0, NT))
    tR = NT - tL
    NW2 = max(lb + tL, rb + tR)

    kpos = [0] * NKC
    qoff = [0] * NKC
    acc = [0, 0]
    pos = [0, 0]
    for k in range(NKC):  # global ascending within each side
        s = sideof[k]
        kpos[k] = pos[s]
        qoff[k] = acc[s]
        pos[s] += 1
        acc[s] += NB[k]
    toff = [lb, rb]  # tail start col within each side
    tlen = [tL, tR]

    return dict(NB=NB, NBAND=NBAND, NT=NT, NW2=NW2, sideof=sideof, kpos=kpos,
                qoff=qoff, toff=toff, tlen=tlen, los=los, his=his, tbs=tbs,
                idxs=idxs)


def make_inputs(X, wq_w, wq_b, wk_w, wk_b, plan):
    X = np.asarray(X, np.float32)
    wq_w = np.asarray(wq_w, np.float32)
    wq_b = np.asarray(wq_b, np.float32)
    wk_w = np.asarray(wk_w, np.float32)
    wk_b = np.asarray(wk_b, np.float32)
    NB, NW2 = plan["NB"], plan["NW2"]
    sideof, kpos, qoff = plan["sideof"], plan["kpos"], plan["qoff"]
    toff, tlen = plan["toff"], plan["tlen"]

    # weights stacked into both partition halves
    wq1 = np.concatenate([wq_w[l] for l in range(NLIN)], axis=1)
    wq = np.concatenate([wq1, wq1], axis=0).astype(BF16)              # (128, 192)
    bq1 = np.stack([wq_b[l] for l in range(NLIN)], axis=1)
    bq = np.concatenate([bq1, bq1], axis=0).astype(np.float32)        # (128, 3)

    in_maps = []
    for c in range(NCORES):
        m, h = c // 2, c % 2
        hs = h * NK
        lo, hi, tb = plan["los"][c], plan["his"][c], plan["tbs"][c]
        idx = plan["idxs"][c]

        qb = np.zeros((2 * D, NW2), BF16)
        msk = np.zeros((C, 2 * NW2), BF16)
        for k in range(NKC):
            n = hi[k] - lo[k]
            s, o = sideof[k], qoff[k]
            if n > 0:
                qb[64 * s:64 * s + 64, o:o + n] = X[0, lo[k]:hi[k], :].T.astype(BF16)
                jg = hs + k * C + np.arange(C)[:, None]
                msk[:, s * NW2 + o:s * NW2 + o + n] = \
                    (jg <= idx[None, lo[k]:hi[k]]).astype(BF16)
        # tail: first tlen[0] tail queries on side 0, rest on side 1
        ntail = T - tb
        n0 = min(ntail, tlen[0])
        if n0 > 0:
            qb[0:64, toff[0]:toff[0] + n0] = X[0, tb:tb + n0, :].T.astype(BF16)
        n1 = ntail - n0
        if n1 > 0:
            qb[64:128, toff[1]:toff[1] + n1] = X[0, tb + n0:, :].T.astype(BF16)

        xk = X[m, hs:hs + NK, :]
        xkt = np.zeros((2 * D, NSIDE * C), BF16)
        v2 = np.zeros((C, 2 * NKC), BF16)
        for k in range(NKC):
            s, p = sideof[k], kpos[k]
            xkt[64 * s:64 * s + 64, p * C:(p + 1) * C] = \
                xk[k * C:(k + 1) * C, :].T.astype(BF16)
            v2[:, 2 * k:2 * k + 2] = xk[k * C:(k + 1) * C, 0:2].astype(BF16)

        wk1 = np.concatenate([wk_w[m, l] for l in range(NLIN)], axis=1)
        wk = np.concatenate([wk1, wk1], axis=0).astype(BF16)          # (128, 192)
        bk1 = np.stack([wk_b[m, l] for l in range(NLIN)], axis=1)
        bk = np.concatenate([bk1, bk1], axis=0).astype(np.float32)    # (128, 3)

        # host-computed S correction: the last K-linear's bias contributes
        # b3 (x) sum_j v2[j,:] per chunk — fold it into the sc prefix inputs
        b3 = wk_b[m, NLIN - 1]                                        # (64,)
        scorr1 = np.zeros((D, 2 * NKC), np.float32)
        for k in range(NKC):
            vs = np.asarray(v2[:, 2 * k:2 * k + 2], np.float32).sum(axis=0)  # (2,)
            scorr1[:, 2 * k:2 * k + 2] = b3[:, None] * vs[None, :]
        scorr = np.concatenate([scorr1, scorr1], axis=0)              # (128, 64)

        in_maps.append(dict(qb=qb, msk=msk, xkt=xkt, v2=v2,
                            wq=wq, bq=bq, wk=wk, bk=bk, scorr=scorr))
    return in_maps


def scatter_outputs(plan, outs):
    """Host-side 'all-reduce': scatter per-core (2, 2*NW2) partials to (T, 2)."""
    NB, NW2 = plan["NB"], plan["NW2"]
    sideof, qoff, toff, tlen = plan["sideof"], plan["qoff"], plan["toff"], plan["tlen"]
    y = np.zeros((T, 2), np.float32)
    for c in range(NCORES):
        o = np.asarray(outs[c], np.float32)
        lo, hi, tb = plan["los"][c], plan["his"][c], plan["tbs"][c]
        for k in range(NKC):
            n = hi[k] - lo[k]
            if n > 0:
                base = sideof[k] * NW2 + qoff[k]
                y[lo[k]:hi[k], :] += o[:, base:base + n].T
        ntail = T - tb
        n0 = min(ntail, tlen[0])
        if n0 > 0:
            y[tb:tb + n0, :] += o[:, toff[0]:toff[0] + n0].T
        n1 = ntail - n0
        if n1 > 0:
            y[tb + n0:, :] += o[:, NW2 + toff[1]:NW2 + toff[1] + n1].T
    return y


# ---------------------------------------------------------------- numpy emulation
def emulate_core(im, plan):
    """Numpy mirror of the device graph for one core (f32 math, for validation)."""
    NB, NW2 = plan["NB"], plan["NW2"]
    sideof, kpos, qoff = plan["sideof"], plan["kpos"], plan["qoff"]
    toff, tlen = plan["toff"], plan["tlen"]

    def f(x):
        return np.asarray(x, np.float32)

    wk, bk, wq, bq = f(im["wk"]), f(im["bk"]), f(im["wq"]), f(im["bq"])
    qb, xkt, v2, msk = f(im["qb"]), f(im["xkt"]), f(im["v2"]), f(im["msk"])

    def mlp_packed(xp, w, b):
        """Row-packed MLP: both halves with their own (identical) weights."""
        a = xp
        outs = []
        for l in range(NLIN):
            z = np.concatenate([
                w[0:64, 64 * l:64 * (l + 1)].T @ a[0:64] + b[0:64, l][:, None],
                w[64:128, 64 * l:64 * (l + 1)].T @ a[64:128] + b[64:128, l][:, None],
            ], axis=0)
            a = np.maximum(z, 0.0) if l < NLIN - 1 else z
            outs.append(a)
        return outs[-1], outs[-2]

    ktp, a2p = mlp_packed(xkt, wk, bk)       # (128, NSIDE*C)
    qtp, _ = mlp_packed(qb, wq, bq)          # (128, NW2)

    # K natural (bias-free) + S chunks (global order) + host bias correction
    sc = np.zeros((64, 2 * NKC), np.float32)
    for k in range(NKC):
        s, p = sideof[k], kpos[k]
        a2 = a2p[64 * s:64 * s + 64, p * C:(p + 1) * C]
        kn = a2.T @ wk[64 * s:64 * s + 64, 128:192]
        sc[:, 2 * k:2 * k + 2] = kn.T @ v2[:, 2 * k:2 * k + 2]
    sc = sc + f(im["scorr"])[0:64]
    srun = np.zeros((64, 2 * (NKC + 1)), np.float32)
    for k in range(NKC):
        srun[:, 2 * k + 2:2 * k + 4] = srun[:, 2 * k:2 * k + 2] + sc[:, 2 * k:2 * k + 2]

    out = np.zeros((2, 2 * NW2), np.float32)
    for k in range(NKC):
        nq = NB[k]
        s, p, o = sideof[k], kpos[k], qoff[k]
        qblk = qtp[64 * s:64 * s + 64, o:o + nq]
        mask = msk[:, s * NW2 + o:s * NW2 + o + nq]
        B = ktp[64 * s:64 * s + 64, p * C:(p + 1) * C].T @ qblk
        out[:, s * NW2 + o:s * NW2 + o + nq] = (
            srun[:, 2 * k:2 * k + 2].T @ qblk + v2[:, 2 * k:2 * k + 2].T @ (B * mask))
    for s in range(2):
        if plan["tlen"][s] > 0:
            o = toff[s]
            out[:, s * NW2 + o:s * NW2 + o + tlen[s]] = \
                srun[:, 2 * NKC:2 * NKC + 2].T @ qtp[64 * s:64 * s + 64, o:o + tlen[s]]
    return out


# ---------------------------------------------------------------- device graph
def build_graph(plan):
    import concourse.bacc as bacc
    import concourse.tile as tile
    from concourse import mybir

    NB, NW2, NT = plan["NB"], plan["NW2"], plan["NT"]
    sideof, kpos, qoff = plan["sideof"], plan["kpos"], plan["qoff"]
    toff, tlen = plan["toff"], plan["tlen"]
    f32 = mybir.dt.float32
    bf16 = mybir.dt.bfloat16
    AF = mybir.ActivationFunctionType
    OP = mybir.AluOpType
    KW = NSIDE * C  # 2048 key cols per side

    nc = bacc.Bacc("TRN2")
    d_qb = nc.dram_tensor("qb", [2 * D, NW2], bf16, kind="ExternalInput")
    d_msk = nc.dram_tensor("msk", [C, 2 * NW2], bf16, kind="ExternalInput")
    d_xkt = nc.dram_tensor("xkt", [2 * D, KW], bf16, kind="ExternalInput")
    d_v2 = nc.dram_tensor("v2", [C, 2 * NKC], bf16, kind="ExternalInput")
    d_wq = nc.dram_tensor("wq", [2 * D, D * NLIN], bf16, kind="ExternalInput")
    d_bq = nc.dram_tensor("bq", [2 * D, NLIN], f32, kind="ExternalInput")
    d_wk = nc.dram_tensor("wk", [2 * D, D * NLIN], bf16, kind="ExternalInput")
    d_bk = nc.dram_tensor("bk", [2 * D, NLIN], f32, kind="ExternalInput")
    d_scorr = nc.dram_tensor("scorr", [C, 2 * NKC], f32, kind="ExternalInput")
    d_out = nc.dram_tensor("out", [2, 2 * NW2], f32, kind="ExternalOutput")

    with ExitStack() as ctx:
        tc = ctx.enter_context(tile.TileContext(nc))
        const = ctx.enter_context(tc.tile_pool(name="const", bufs=1))
        big = ctx.enter_context(tc.tile_pool(name="big", bufs=1))
        work = ctx.enter_context(tc.tile_pool(name="work", bufs=3))
        pmlp = ctx.enter_context(tc.tile_pool(name="pmlp", bufs=3, space="PSUM"))
        ps = ctx.enter_context(tc.tile_pool(name="ps", bufs=1, space="PSUM"))
        pb = ctx.enter_context(tc.tile_pool(name="pb", bufs=2, space="PSUM"))
        pout = ctx.enter_context(tc.tile_pool(name="pout", bufs=2, space="PSUM"))

        wq_t = const.tile([2 * D, D * NLIN], bf16, tag="wq")
        bq_t = const.tile([2 * D, NLIN], f32, tag="bq")
        wk_t = const.tile([2 * D, D * NLIN], bf16, tag="wk")
        bk_t = const.tile([2 * D, NLIN], f32, tag="bk")
        scorr_t = const.tile([C, 2 * NKC], f32, tag="scorr")
        v2_t = const.tile([C, 2 * NKC], bf16, tag="v2")

        # compute-critical loads first, spread over the two HWDGE engines
        nc.sync.dma_start(wk_t[:], d_wk[:])
        nc.scalar.dma_start(bk_t[:], d_bk[:])
        xkt_t = big.tile([2 * D, KW], bf16, tag="xkt")
        nc.sync.dma_start(xkt_t[:], d_xkt[:])
        nc.scalar.dma_start(wq_t[:], d_wq[:])
        nc.scalar.dma_start(bq_t[:], d_bq[:])
        qb_t = big.tile([2 * D, NW2], bf16, tag="qb")
        NQB = 4
        qsp = _round_up((NW2 + NQB - 1) // NQB, 8)
        for i in range(NQB):
            a, b = i * qsp, min((i + 1) * qsp, NW2)
            if a < b:
                nc.sync.dma_start(qb_t[:, a:b], d_qb[:, a:b])
        nc.scalar.dma_start(v2_t[:], d_v2[:])
        nc.scalar.dma_start(scorr_t[:], d_scorr[:])
        msk_t = big.tile([C, 2 * NW2], bf16, tag="msk")
        NMQ = 4
        msp = _round_up((2 * NW2 + NMQ - 1) // NMQ, 8)
        for i in range(NMQ):
            a, b = i * msp, min((i + 1) * msp, 2 * NW2)
            if a < b:
                nc.scalar.dma_start(msk_t[:, a:b], d_msk[:, a:b])

        kt_t = big.tile([2 * D, KW], bf16, tag="kt")
        a2k_t = big.tile([2 * D, KW], bf16, tag="a2k")
        qt_t = big.tile([2 * D, NW2], bf16, tag="qt")
        kn_t = big.tile([C, D * NKC], bf16, tag="kn")
        sc_t = big.tile([C, 2 * NKC], f32, tag="sc")
        srun_t = big.tile([C, 2 * (NKC + 1)], f32, tag="srun")
        srunb_t = big.tile([C, 2 * (NKC + 1)], bf16, tag="srunb")
        outs_t = big.tile([2, 2 * NW2], f32, tag="outs")

        # PE warm-up burst: ~4us of dense dummy matmuls during the initial DMA
        # window flips the HAM clock gate to 8/8 before real work arrives
        wup_t = work.tile([C, FMAX], bf16, tag="wup", name="wup")
        nc.vector.memset(wup_t[:], 0.0)
        for _ in range(12):
            pwu = pb.tile([C, FMAX], f32, tag="pb", name="pwu")
            nc.tensor.matmul(pwu[:], wup_t[:, 0:C], wup_t[:], start=True, stop=True)

        def mlp3(src_t, w_t, b_t, n_cols, out_t, mid_t):
            """Row-packed 3-layer MLP, layer-major: per layer, stream all column
            blocks through the PE back-to-back (same stationary weights), with
            bias+relu epilogues alternating between DVE and ACT."""
            nblk = (n_cols + FMAX - 1) // FMAX
            stage = [src_t, None, None, None]
            stage[1] = work.tile([C, n_cols], bf16, tag="mlpa", name="mlpa")
            stage[2] = mid_t
            stage[3] = out_t
            for l in range(NLIN):
                for bi in range(nblk):
                    a = bi * FMAX
                    b = min(a + FMAX, n_cols)
                    n = b - a
                    cur = stage[l][:, a:b]
                    pz = pmlp.tile([C, FMAX], f32, tag="pmlp", name="pz")
                    nc.tensor.matmul(pz[0:64, :n], w_t[0:64, D * l:D * (l + 1)],
                                     cur[0:64, :], start=True, stop=True,
                                     tile_position=(0, 0))
                    nc.tensor.matmul(pz[64:128, :n], w_t[64:128, D * l:D * (l + 1)],
                                     cur[64:128, :], start=True, stop=True,
                                     tile_position=(64, 64))
                    dst = stage[l + 1][:, a:b]
                    if bi % 2 == 0:
                        if l < NLIN - 1:
                            nc.vector.tensor_scalar(dst, pz[:, :n],
                                                    b_t[:, l:l + 1], 0.0,
                                                    OP.add, OP.max)
                        else:
                            nc.vector.tensor_scalar_add(dst, pz[:, :n],
                                                        b_t[:, l:l + 1])
                    else:
                        nc.scalar.activation(dst, pz[:, :n],
                                             AF.Relu if l < NLIN - 1 else AF.Identity,
                                             bias=b_t[:, l:l + 1])

        # K mlp
        mlp3(xkt_t, wk_t, bk_t, KW, kt_t, a2k_t)

        # K natural per chunk (bias-free; bias folded into scorr): full 128-part out
        for k in range(NKC):
            s, p = sideof[k], kpos[k]
            pkn = pb.tile([C, FMAX], f32, tag="pb", name="pkn")
            nc.tensor.matmul(pkn[:, :D], a2k_t[64 * s:64 * s + 64, C * p:C * (p + 1)],
                             wk_t[64 * s:64 * s + 64, 2 * D:3 * D],
                             start=True, stop=True, tile_position=(64 * s, 0))
            nc.scalar.copy(kn_t[:, D * k:D * (k + 1)], pkn[:, :D])

        # S chunks, duplicated into both partition halves of one PSUM tile
        psc = ps.tile([C, 2 * NKC], f32, tag="ps", name="psc")
        for k in range(NKC):
            nc.tensor.matmul(psc[0:64, 2 * k:2 * k + 2], kn_t[:, D * k:D * (k + 1)],
                             v2_t[:, 2 * k:2 * k + 2], start=True, stop=True,
                             tile_position=(0, 0))
            nc.tensor.matmul(psc[64:128, 2 * k:2 * k + 2], kn_t[:, D * k:D * (k + 1)],
                             v2_t[:, 2 * k:2 * k + 2], start=True, stop=True,
                             tile_position=(0, 64))
        nc.vector.tensor_add(sc_t[:], psc[:], scorr_t[:])

        # prefix sums (f32, both halves at once) + bf16 copy for lhsT use
        nc.vector.memset(srun_t[:, 0:2], 0.0)
        for k in range(NKC):
            nc.vector.tensor_add(srun_t[:, 2 * k + 2:2 * k + 4],
                                 srun_t[:, 2 * k:2 * k + 2], sc_t[:, 2 * k:2 * k + 2])
        nc.scalar.copy(srunb_t[:], srun_t[:])

        # Q mlp (mid shares qt_t — Tile serializes the per-block WAR)
        mlp3(qb_t, wq_t, bq_t, NW2, qt_t, qt_t)

        # band chunks, ordered by (side, column offset) for locality
        korder = sorted(range(NKC), key=lambda k: (sideof[k], qoff[k]))
        for k in korder:
            nq = NB[k]
            s, p, o0 = sideof[k], kpos[k], int(qoff[k])
            for a in range(0, nq, FMAX):
                b = min(a + FMAX, nq)
                n = b - a
                qs = qt_t[64 * s:64 * s + 64, o0 + a:o0 + b]
                pB = pb.tile([C, FMAX], f32, tag="pb", name="pB")
                nc.tensor.matmul(pB[:, :n], kt_t[64 * s:64 * s + 64, C * p:C * (p + 1)],
                                 qs, start=True, stop=True, tile_position=(64 * s, 0))
                bm = work.tile([C, FMAX], bf16, tag="bm", name="bm")
                nc.vector.tensor_mul(bm[:, :n], pB[:, :n],
                                     msk_t[:, s * NW2 + o0 + a:s * NW2 + o0 + b])
                po = pout.tile([2, FMAX], f32, tag="pout", name="po")
                nc.tensor.matmul(po[:, :n], srunb_t[64 * s:64 * s + 64, 2 * k:2 * k + 2],
                                 qs, start=True, stop=False, tile_position=(64 * s, 0))
                nc.tensor.matmul(po[:, :n], v2_t[:, 2 * k:2 * k + 2], bm[:, :n],
                                 start=False, stop=True, tile_position=(0, 0))
                nc.scalar.copy(outs_t[:, s * NW2 + o0 + a:s * NW2 + o0 + b], po[:, :n])

        # tails (state-only), per side
        for s in range(2):
            for a in range(0, tlen[s], FMAX):
                b = min(a + FMAX, tlen[s])
                n = b - a
                po = pout.tile([2, FMAX], f32, tag="pout", name="po_t")
                nc.tensor.matmul(po[:, :n],
                                 srunb_t[64 * s:64 * s + 64, 2 * NKC:2 * NKC + 2],
                                 qt_t[64 * s:64 * s + 64, toff[s] + a:toff[s] + b],
                                 start=True, stop=True, tile_position=(64 * s, 0))
                nc.scalar.copy(outs_t[:, s * NW2 + toff[s] + a:s * NW2 + toff[s] + b],
                               po[:, :n])

        nc.sync.dma_start(d_out[:], outs_t[:])

    nc.finalize()
    return nc


_CACHE = {}


def kernel(X, wq_w, wq_b, wk_w, wk_b):
    from concourse.bass_utils import run_bass_kernel_spmd

    plan = make_plan(X)
    in_maps = make_inputs(X, wq_w, wq_b, wk_w, wk_b, plan)
    key = (tuple(plan["NB"]), plan["NT"], tuple(plan["sideof"]))
    if key not in _CACHE:
        _CACHE[key] = build_graph(plan)
    nc = _CACHE[key]
    res = run_bass_kernel_spmd(nc, in_maps, core_ids=list(range(NCORES)),
                               trace=bool(int(os.environ.get("KTRACE", "0"))))
    outs = [res.results[c]["out"] for c in range(NCORES)]
    y = scatter_outputs(plan, outs)
    if os.environ.get("KTRACE", "0") != "0":
        kernel.last_result = res
    return y[None]  # (1, T, 2)
